# revision 7
# baseline (speedup 1.0000x reference)
"""Trainium2 Bass kernel for MAS-LoRA linear (moe_routing).

Reference computation (per batch element b):
    out[b] = x[b] @ W_base.T + b_base
             + SCALING * sum_e w[b,e] * (x[b] @ As[e].T) @ Bs[e].T

Strategy: data-parallel over batch across 8 cores (2 batch elements per
core).  The LoRA term is folded on the HOST into an effective weight per
batch element
    W_eff[b] = W_base + SCALING * sum_e w[b,e] * (Bs[e] @ As[e])
(a cheap rank-128 update, ~0.3 GFLOP total on host), so the device kernel
is a pure batched GEMM
    outT[o, t] = sum_c W_eff.T[c, o] * xT[c, t] + b_base[o]
computed transposed (tokens moving, weights stationary) in bf16 (fp32 PSUM
accumulation), with the bias applied during PSUM->SBUF eviction and bf16
stores upcast on the host.  Host transposes/conversions are part of the
shard/unshard step.
"""

import numpy as np
import ml_dtypes

import concourse.bass as bass
import concourse.mybir as mybir
import concourse.tile as tile
from concourse.bass_utils import run_bass_kernel_spmd

FP32 = mybir.dt.float32
BF16 = mybir.dt.bfloat16
NP_BF16 = ml_dtypes.bfloat16

# Problem shapes (hardcoded per contract)
B, T, C, O, E, R = 16, 1500, 1024, 1024, 8, 16
ER = E * R  # 128
SCALING = 32.0 / 16.0  # alpha / r = 2.0
NCORES = 8
BPC = B // NCORES       # batch elems per core = 2
TPC = BPC * T           # tokens per core = 3000
CT = C // 128           # 8 c tiles
OT = O // 128           # 8 o tiles

_counter = [0]


def _split_multi_waits(nc):
    """This walrus build supports one sync-wait command per instruction;
    Tile can emit several.  Hoist extras onto single-wait NoOps just before
    the instruction (same engine => identical semantics)."""
    for fn in nc.m.functions:
        for blk in fn.blocks:
            insts = blk.instructions
            if not any(
                i.sync_info and len(i.sync_info.on_wait) > 1 for i in insts
            ):
                continue
            out = []
            for inst in insts:
                si = inst.sync_info
                if si is not None and len(si.on_wait) > 1:
                    waits = list(si.on_wait)
                    for w in waits[:-1]:
                        _counter[0] += 1
                        out.append(
                            mybir.InstNoOp(
                                name=f"waitsplit-{_counter[0]}",
                                engine=inst.engine,
                                ins=[],
                                outs=[],
                                sync_info=mybir.SyncInfo(on_wait=[w], on_update=[]),
                            )
                        )
                    si.on_wait = [waits[-1]]
                out.append(inst)
            blk.instructions = out
    return nc


def build_nc(split=True, n_iter=1, serial=False, n_warm=7,
             cs_first=(476, 512, 512), cs_last=(512, 476, 256, 256),
             pso_bufs=8, xin_bufs=3, out_bufs=3):
    nc = bass.Bass()
    xT_d = nc.declare_dram_parameter("xT", [C, TPC], BF16, isOutput=False)
    W_d = nc.declare_dram_parameter("Weff", [BPC, C, O], BF16, isOutput=False)
    bcol_d = nc.declare_dram_parameter("bcol", [128, OT], FP32, isOutput=False)
    outT_d = nc.declare_dram_parameter("outT", [O, TPC], BF16, isOutput=True)

    xT_r = xT_d.rearrange("(ct cp) t -> cp ct t", cp=128)
    W_r = W_d.rearrange("b (ct cp) o -> cp b ct o", cp=128)
    outT_r = outT_d.rearrange("(ot op) t -> op ot t", op=128)

    with tile.TileContext(nc) as tc:
        with (
            tc.tile_pool(name="const", bufs=1) as constp,
            tc.tile_pool(name="weff", bufs=2 * CT) as weffp,
            tc.tile_pool(name="xin", bufs=xin_bufs) as xinp,
            tc.tile_pool(name="outs", bufs=out_bufs) as outp,
            tc.tile_pool(name="pso", bufs=pso_bufs, space="PSUM") as psop,
        ):
            # PE clock (HAM) warmup on dummy data so early GEMM matmuls run
            # at full speed; depends on no DMA (memset on Pool).
            warm = constp.tile([128, 512], BF16)
            nc.gpsimd.memset(warm[:], 0.0)
            pwu = psop.tile([128, 512], FP32, tag="pso", name="pwu")
            for _ in range(n_warm):
                nc.tensor.matmul(
                    pwu[:], warm[:, 0:128], warm[:], start=True, stop=True
                )

            # W_eff b=0 tiles, alternating scalar/vector queues
            wt = [
                [
                    weffp.tile([128, O], BF16, tag="wt", name=f"wt{b}_{ct}")
                    for ct in range(CT)
                ]
                for b in range(BPC)
            ]
            for ct in range(CT):
                nc.scalar.dma_start(wt[0][ct][:], W_r[:, 0, ct, :])

            # first x chunk in 4 ct-pair pieces on sync+pool for fast arrival
            cs0 = cs_first[0]
            xt0 = xinp.tile([128, CT, cs0], BF16, tag="xt", name="xt_pre")
            nc.sync.dma_start(xt0[:, 0:2, :], xT_r[:, 0:2, 0:cs0])
            nc.gpsimd.dma_start(xt0[:, 2:4, :], xT_r[:, 2:4, 0:cs0])
            nc.sync.dma_start(xt0[:, 4:6, :], xT_r[:, 4:6, 0:cs0])
            nc.gpsimd.dma_start(xt0[:, 6:8, :], xT_r[:, 6:8, 0:cs0])

            bcol_sb = constp.tile([128, OT], FP32)
            nc.gpsimd.dma_start(bcol_sb[:], bcol_d[:])

            plans = {0: list(cs_first), BPC - 1: list(cs_last)}
            for it in range(n_iter):
                if serial and it > 0:
                    tc.strict_bb_all_engine_barrier()
                for b in range(BPC):
                    plan = plans.get(b, [512, 512, 476])
                    assert sum(plan) == T
                    plan_off = [b * T + sum(plan[:i]) for i in range(len(plan))]
                    for ch, csz in enumerate(plan):
                        t0 = plan_off[ch]
                        is_last_chunk = (
                            it == n_iter - 1
                            and b == BPC - 1
                            and ch == len(plan) - 1
                        )
                        if it == 0 and b == 0 and ch == 0:
                            xt = xt0
                        else:
                            xt = xinp.tile([128, CT, csz], BF16, tag="xt")
                            nc.sync.dma_start(xt[:], xT_r[:, :, t0 : t0 + csz])

                        osb = outp.tile([128, OT, csz], BF16, tag="osb")
                        psos = [
                            psop.tile(
                                [128, csz], FP32, tag="pso",
                                name=f"pso{it}_{b}_{ch}_{ot}",
                            )
                            for ot in range(OT)
                        ]

                        def evict(ot, eng_act):
                            if eng_act:
                                nc.scalar.activation(
                                    osb[:, ot, :],
                                    psos[ot][:],
                                    mybir.ActivationFunctionType.Identity,
                                    bias=bcol_sb[:, ot : ot + 1],
                                )
                            else:
                                nc.vector.tensor_scalar_add(
                                    osb[:, ot, :], psos[ot][:],
                                    bcol_sb[:, ot : ot + 1],
                                )

                        if is_last_chunk:
                            # ot-outer: drain evict+store for ot<7 during the
                            # GEMM so only ot=7's chain trails the last matmul
                            for ot in range(OT):
                                for ct in range(CT):
                                    nc.tensor.matmul(
                                        psos[ot][:],
                                        wt[b][ct][:, ot * 128 : (ot + 1) * 128],
                                        xt[:, ct, :],
                                        start=(ct == 0),
                                        stop=(ct == CT - 1),
                                    )
                                # final ot: Act evict (cheapest), store alone
                                # on SP; earlier ots: alternate evict, pool st
                                evict(ot, eng_act=(ot % 2 == 0 or ot == OT - 1))
                                eng = nc.sync if ot == OT - 1 else nc.gpsimd
                                eng.dma_start(
                                    outT_r[:, ot : ot + 1, t0 : t0 + csz],
                                    osb[:, ot : ot + 1, :],
                                )
                        else:
                            # ct-outer so all 8 psos accumulate as tiles land
                            for ct in range(CT):
                                for ot in range(OT):
                                    nc.tensor.matmul(
                                        psos[ot][:],
                                        wt[b][ct][:, ot * 128 : (ot + 1) * 128],
                                        xt[:, ct, :],
                                        start=(ct == 0),
                                        stop=(ct == CT - 1),
                                    )
                            for ot in range(OT):
                                evict(ot, eng_act=(ot % 2 == 0))
                                if ot % 2 == 1:
                                    # per-2-ot stores on pool
                                    nc.gpsimd.dma_start(
                                        outT_r[:, ot - 1 : ot + 1, t0 : t0 + csz],
                                        osb[:, ot - 1 : ot + 1, :],
                                    )
                        if it == 0 and b == 0 and ch == 0:
                            # W_eff b=1 tiles after chunk0 work is queued
                            for ct in range(CT):
                                nc.scalar.dma_start(wt[1][ct][:], W_r[:, 1, ct, :])

    if split:
        _split_multi_waits(nc)
    return nc


_cache = {}


BEST = dict(
    n_warm=1,
    cs_first=(476, 512, 512),
    cs_last=(512, 476, 256, 256),
)


def _get_nc():
    if "nc" not in _cache:
        _cache["nc"] = build_nc(**BEST)
    return _cache["nc"]


def host_prep(x, w, W_base, b_base, As, Bs):
    """Fold the LoRA term into per-batch effective weights and lay out all
    device inputs (transposed / bf16)."""
    x = np.asarray(x, dtype=np.float32)
    w = np.asarray(w, dtype=np.float32)
    W_base = np.asarray(W_base, dtype=np.float32)
    b_base = np.asarray(b_base, dtype=np.float32)
    As = np.asarray(As, dtype=np.float32)
    Bs = np.asarray(Bs, dtype=np.float32)

    BA = np.matmul(Bs, As)                                   # [E, O, C]
    D = np.tensordot(w, BA.reshape(E, -1), ([1], [0]))       # [B, O*C]
    Weff = W_base.reshape(1, O, C) + SCALING * D.reshape(B, O, C)
    WeffT = np.ascontiguousarray(Weff.transpose(0, 2, 1)).astype(NP_BF16)
    bcol = np.ascontiguousarray(b_base.reshape(OT, 128).T)   # [op, ot]

    in_maps = []
    for i in range(NCORES):
        xs = x[i * BPC : (i + 1) * BPC].reshape(TPC, C)
        xT_i = np.ascontiguousarray(xs.T).astype(NP_BF16)    # [c, t]
        in_maps.append(
            {
                "xT": xT_i,
                "Weff": WeffT[i * BPC : (i + 1) * BPC],
                "bcol": bcol,
            }
        )
    return in_maps


def kernel(x, w, W_base, b_base, As, Bs, trace=False):
    in_maps = host_prep(x, w, W_base, b_base, As, Bs)

    nc = _get_nc()
    res = run_bass_kernel_spmd(nc, in_maps, list(range(NCORES)), trace=trace)

    out = np.empty((B, T, O), dtype=np.float32)
    for i in range(NCORES):
        outT_i = np.asarray(res.results[i]["outT"]).astype(np.float32)  # [o, t]
        out[i * BPC : (i + 1) * BPC] = outT_i.T.reshape(BPC, T, O)

    if trace:
        kernel.last_result = res
    return out


# revision 13
# speedup vs baseline: 1.0529x; 1.0529x over previous
"""Trainium2 Bass kernel for MAS-LoRA linear (moe_routing).

Reference computation (per batch element b):
    out[b] = x[b] @ W_base.T + b_base
             + SCALING * sum_e w[b,e] * (x[b] @ As[e].T) @ Bs[e].T

Strategy: data-parallel over batch across 8 cores (2 batch elements per
core).  The LoRA term is folded on the HOST into an effective weight per
batch element
    W_eff[b] = W_base + SCALING * sum_e w[b,e] * (Bs[e] @ As[e])
(a cheap rank-128 update, ~0.3 GFLOP total on host), so the device kernel
is a pure batched GEMM
    outT[o, t] = sum_c W_eff.T[c, o] * xT[c, t] + b_base[o]
computed transposed (tokens moving, weights stationary) in bf16 (fp32 PSUM
accumulation), with the bias applied during PSUM->SBUF eviction and bf16
stores upcast on the host.  Host transposes/conversions are part of the
shard/unshard step.
"""

import numpy as np
import ml_dtypes

import concourse.bass as bass
import concourse.mybir as mybir
import concourse.tile as tile
from concourse.bass_utils import run_bass_kernel_spmd

FP32 = mybir.dt.float32
BF16 = mybir.dt.bfloat16
NP_BF16 = ml_dtypes.bfloat16

# Problem shapes (hardcoded per contract)
B, T, C, O, E, R = 16, 1500, 1024, 1024, 8, 16
ER = E * R  # 128
SCALING = 32.0 / 16.0  # alpha / r = 2.0
NCORES = 8
BPC = B // NCORES       # batch elems per core = 2
TPC = BPC * T           # tokens per core = 3000
CT = C // 128           # 8 c tiles
OT = O // 128           # 8 o tiles

_counter = [0]


def _split_multi_waits(nc):
    """This walrus build supports one sync-wait command per instruction;
    Tile can emit several.  Hoist extras onto single-wait NoOps just before
    the instruction (same engine => identical semantics)."""
    for fn in nc.m.functions:
        for blk in fn.blocks:
            insts = blk.instructions
            if not any(
                i.sync_info and len(i.sync_info.on_wait) > 1 for i in insts
            ):
                continue
            out = []
            for inst in insts:
                si = inst.sync_info
                if si is not None and len(si.on_wait) > 1:
                    waits = list(si.on_wait)
                    for w in waits[:-1]:
                        _counter[0] += 1
                        out.append(
                            mybir.InstNoOp(
                                name=f"waitsplit-{_counter[0]}",
                                engine=inst.engine,
                                ins=[],
                                outs=[],
                                sync_info=mybir.SyncInfo(on_wait=[w], on_update=[]),
                            )
                        )
                    si.on_wait = [waits[-1]]
                out.append(inst)
            blk.instructions = out
    return nc


def build_nc(split=True, n_iter=1, serial=False, n_warm=7, warm_cols=512,
             cs_first=(476, 512, 512), cs_last=(512, 476, 256, 256),
             pso_bufs=8, xin_bufs=3, out_bufs=3, split_first=False,
             last_fin=64):
    nc = bass.Bass()
    xT_d = nc.declare_dram_parameter("xT", [C, TPC], BF16, isOutput=False)
    W_d = nc.declare_dram_parameter("Weff", [BPC, C, O], BF16, isOutput=False)
    bcol_d = nc.declare_dram_parameter("bcol", [128, OT], FP32, isOutput=False)
    outT_d = nc.declare_dram_parameter("outT", [O, TPC], BF16, isOutput=True)

    xT_r = xT_d.rearrange("(ct cp) t -> cp ct t", cp=128)
    W_r = W_d.rearrange("b (ct cp) o -> cp b ct o", cp=128)
    outT_r = outT_d.rearrange("(ot op) t -> op ot t", op=128)

    with tile.TileContext(nc) as tc:
        with (
            tc.tile_pool(name="const", bufs=1) as constp,
            tc.tile_pool(name="weff", bufs=2 * CT) as weffp,
            tc.tile_pool(name="xin", bufs=xin_bufs) as xinp,
            tc.tile_pool(name="outs", bufs=out_bufs) as outp,
            tc.tile_pool(name="pso", bufs=pso_bufs, space="PSUM") as psop,
        ):
            # PE clock (HAM) warmup on dummy data so early GEMM matmuls run
            # at full speed; depends on no DMA (memset on Pool).
            warm = constp.tile([128, warm_cols], BF16)
            nc.gpsimd.memset(warm[:], 0.0)
            pwu = psop.tile([128, warm_cols], FP32, tag="pso", name="pwu")
            for _ in range(n_warm):
                nc.tensor.matmul(
                    pwu[:], warm[:, 0:128], warm[:], start=True, stop=True
                )

            # W_eff b=0 tiles, alternating scalar/vector queues
            wt = [
                [
                    weffp.tile([128, O], BF16, tag="wt", name=f"wt{b}_{ct}")
                    for ct in range(CT)
                ]
                for b in range(BPC)
            ]
            if split_first:
                # halve the first weff tile load so ot 0-3 can start sooner
                nc.scalar.dma_start(wt[0][0][:, 0:512], W_r[:, 0, 0, 0:512])
                nc.scalar.dma_start(wt[0][0][:, 512:O], W_r[:, 0, 0, 512:O])
                for ct in range(1, CT):
                    nc.scalar.dma_start(wt[0][ct][:], W_r[:, 0, ct, :])
            else:
                for ct in range(CT):
                    nc.scalar.dma_start(wt[0][ct][:], W_r[:, 0, ct, :])

            # first x chunk in ct pieces on sync+pool for fast arrival
            cs0 = cs_first[0]
            xt0 = xinp.tile([128, CT, cs0], BF16, tag="xt", name="xt_pre")
            if split_first:
                nc.sync.dma_start(xt0[:, 0:1, :], xT_r[:, 0:1, 0:cs0])
                nc.sync.dma_start(xt0[:, 1:2, :], xT_r[:, 1:2, 0:cs0])
                nc.gpsimd.dma_start(xt0[:, 2:4, :], xT_r[:, 2:4, 0:cs0])
                nc.sync.dma_start(xt0[:, 4:6, :], xT_r[:, 4:6, 0:cs0])
                nc.gpsimd.dma_start(xt0[:, 6:8, :], xT_r[:, 6:8, 0:cs0])
            else:
                nc.sync.dma_start(xt0[:, 0:2, :], xT_r[:, 0:2, 0:cs0])
                nc.gpsimd.dma_start(xt0[:, 2:4, :], xT_r[:, 2:4, 0:cs0])
                nc.sync.dma_start(xt0[:, 4:6, :], xT_r[:, 4:6, 0:cs0])
                nc.gpsimd.dma_start(xt0[:, 6:8, :], xT_r[:, 6:8, 0:cs0])

            bcol_sb = constp.tile([128, OT], FP32)
            nc.gpsimd.dma_start(bcol_sb[:], bcol_d[:])

            plans = {0: list(cs_first), BPC - 1: list(cs_last)}
            for it in range(n_iter):
                if serial and it > 0:
                    tc.strict_bb_all_engine_barrier()
                for b in range(BPC):
                    plan = plans.get(b, [512, 512, 476])
                    assert sum(plan) == T
                    plan_off = [b * T + sum(plan[:i]) for i in range(len(plan))]
                    for ch, csz in enumerate(plan):
                        t0 = plan_off[ch]
                        is_last_chunk = (
                            it == n_iter - 1
                            and b == BPC - 1
                            and ch == len(plan) - 1
                        )
                        if it == 0 and b == 0 and ch == 0:
                            xt = xt0
                        else:
                            xt = xinp.tile([128, CT, csz], BF16, tag="xt")
                            nc.sync.dma_start(xt[:], xT_r[:, :, t0 : t0 + csz])

                        osb = outp.tile([128, OT, csz], BF16, tag="osb")
                        psos = [
                            psop.tile(
                                [128, csz], FP32, tag="pso",
                                name=f"pso{it}_{b}_{ch}_{ot}",
                            )
                            for ot in range(OT)
                        ]

                        def evict(ot, eng_act):
                            if eng_act:
                                nc.scalar.activation(
                                    osb[:, ot, :],
                                    psos[ot][:],
                                    mybir.ActivationFunctionType.Identity,
                                    bias=bcol_sb[:, ot : ot + 1],
                                )
                            else:
                                nc.vector.tensor_scalar_add(
                                    osb[:, ot, :], psos[ot][:],
                                    bcol_sb[:, ot : ot + 1],
                                )

                        if is_last_chunk:
                            # ot-outer: drain evict+store for ot<7 during the
                            # GEMM so only ot=7's chain trails the last matmul
                            for ot in range(OT - 1):
                                for ct in range(CT):
                                    nc.tensor.matmul(
                                        psos[ot][:],
                                        wt[b][ct][:, ot * 128 : (ot + 1) * 128],
                                        xt[:, ct, :],
                                        start=(ct == 0),
                                        stop=(ct == CT - 1),
                                    )
                                evict(ot, eng_act=(ot % 2 == 0))
                                nc.gpsimd.dma_start(
                                    outT_r[:, ot : ot + 1, t0 : t0 + csz],
                                    osb[:, ot : ot + 1, :],
                                )
                            # final ot in two token-halves so only a tiny
                            # evict+store chain trails the very last matmul
                            ot = OT - 1
                            fin = last_fin
                            for piece, (p0, p1) in enumerate(
                                ((0, csz - fin), (csz - fin, csz))
                            ):
                                psoh = psop.tile(
                                    [128, p1 - p0], FP32, tag="pso",
                                    name=f"psoh{it}_{piece}",
                                )
                                for ct in range(CT):
                                    nc.tensor.matmul(
                                        psoh[:],
                                        wt[b][ct][:, ot * 128 : (ot + 1) * 128],
                                        xt[:, ct, p0:p1],
                                        start=(ct == 0),
                                        stop=(ct == CT - 1),
                                    )
                                nc.scalar.activation(
                                    osb[:, ot, p0:p1],
                                    psoh[:],
                                    mybir.ActivationFunctionType.Identity,
                                    bias=bcol_sb[:, ot : ot + 1],
                                )
                                eng = nc.sync if piece else nc.gpsimd
                                eng.dma_start(
                                    outT_r[:, ot, t0 + p0 : t0 + p1],
                                    osb[:, ot, p0:p1],
                                )
                        else:
                            # ct-outer so all 8 psos accumulate as tiles land
                            for ct in range(CT):
                                for ot in range(OT):
                                    nc.tensor.matmul(
                                        psos[ot][:],
                                        wt[b][ct][:, ot * 128 : (ot + 1) * 128],
                                        xt[:, ct, :],
                                        start=(ct == 0),
                                        stop=(ct == CT - 1),
                                    )
                            for ot in range(OT):
                                evict(ot, eng_act=(ot % 2 == 0))
                                if ot % 2 == 1:
                                    # per-2-ot stores on pool
                                    nc.gpsimd.dma_start(
                                        outT_r[:, ot - 1 : ot + 1, t0 : t0 + csz],
                                        osb[:, ot - 1 : ot + 1, :],
                                    )
                        if it == 0 and b == 0 and ch == 0:
                            # W_eff b=1 tiles after chunk0 work is queued
                            for ct in range(CT):
                                nc.scalar.dma_start(wt[1][ct][:], W_r[:, 1, ct, :])

    if split:
        _split_multi_waits(nc)
    return nc


_cache = {}


BEST = dict(
    n_warm=2,
    warm_cols=256,
    split_first=True,
    last_fin=128,
    cs_first=(476, 512, 512),
    cs_last=(512, 476, 256, 256),
)


def _get_nc():
    if "nc" not in _cache:
        _cache["nc"] = build_nc(**BEST)
    return _cache["nc"]


def host_prep(x, w, W_base, b_base, As, Bs):
    """Fold the LoRA term into per-batch effective weights and lay out all
    device inputs (transposed / bf16)."""
    x = np.asarray(x, dtype=np.float32)
    w = np.asarray(w, dtype=np.float32)
    W_base = np.asarray(W_base, dtype=np.float32)
    b_base = np.asarray(b_base, dtype=np.float32)
    As = np.asarray(As, dtype=np.float32)
    Bs = np.asarray(Bs, dtype=np.float32)

    BA = np.matmul(Bs, As)                                   # [E, O, C]
    D = np.tensordot(w, BA.reshape(E, -1), ([1], [0]))       # [B, O*C]
    Weff = W_base.reshape(1, O, C) + SCALING * D.reshape(B, O, C)
    WeffT = np.ascontiguousarray(Weff.transpose(0, 2, 1)).astype(NP_BF16)
    bcol = np.ascontiguousarray(b_base.reshape(OT, 128).T)   # [op, ot]

    in_maps = []
    for i in range(NCORES):
        xs = x[i * BPC : (i + 1) * BPC].reshape(TPC, C)
        xT_i = np.ascontiguousarray(xs.T).astype(NP_BF16)    # [c, t]
        in_maps.append(
            {
                "xT": xT_i,
                "Weff": WeffT[i * BPC : (i + 1) * BPC],
                "bcol": bcol,
            }
        )
    return in_maps


def kernel(x, w, W_base, b_base, As, Bs, trace=False):
    in_maps = host_prep(x, w, W_base, b_base, As, Bs)

    nc = _get_nc()
    res = run_bass_kernel_spmd(nc, in_maps, list(range(NCORES)), trace=trace)

    out = np.empty((B, T, O), dtype=np.float32)
    for i in range(NCORES):
        outT_i = np.asarray(res.results[i]["outT"]).astype(np.float32)  # [o, t]
        out[i * BPC : (i + 1) * BPC] = outT_i.T.reshape(BPC, T, O)

    if trace:
        kernel.last_result = res
    return out
